# revision 55
# baseline (speedup 1.0000x reference)
"""Trainium2 Bass kernel for nn_Actor (3 grouped conv1d blocks + dense + tanh).

Sharding: column-parallel across 8 cores. Core j owns input channels
{2j, 2j+1}; every conv is grouped (depthwise x8 filters), so that
slice owns contiguous channel blocks through the whole net:
  conv1 out-ch [16j,16j+16), conv2 out-ch [128j,128j+128),
  conv3 out-ch [1024j, 1024j+1024), and rows {l*8192 + ch} of W.
Each core computes partial dense outputs; the host sums them, adds bd
and applies tanh.

Pipeline design (all bf16 compute, f32 psum):
- conv1 out tmp1 [128p = (c1*8 + bg), (l1, b8)].
- i2 [80p = (c1*5 + k), (bg, l2, b8)] via 8 DMAs (one per bg).
- x2r [128p = P2(c2), (bg, l2, b8)] 960-stride cols.
- conv3 im2col as half-group tiles i3 [80p = (c*5+k), 4x928]:
  i3[(c,k), bg*928+col] = x2r[P(c), bg*960 + col + 8k] (tap shift is a
  column offset, so each (partition, tap, bg) is one 1856B contiguous
  segment; 10 DMA instructions per half, 16 tile bufs so every issue's
  buffer WAR is ancient and the issuing rings never block; the e=0 half
  rides the SP ring and e=1 the Pool ring, so concurrent transfers never
  contend on the same SBUF source ports).
- conv3: per (group, bg): 2 MMs (512+416) into a [128, 1024] 2-bank psum
  tile (3-buf rotation), ONE relu [128, 928] per tile (alternating
  Act/DVE) -> packed zf [128, 8*928] (bufs=3).
- dense: quad-MMs reading zf(g-2) (2-group-old => RAW waits ancient),
  interleaved ~4 per bg slot BEFORE the chunk MMs so PE gate-waits are
  covered by wait-free MMs; 4 col-tile accumulators in ONE psum bank;
  group 6's quads fold into group 7's slots, only group 7 tails.
- per-iteration software pipelining (For_i has an all-engine barrier, but
  DMA queues run through it): the NEXT pass's conv1/i2/conv2 + i3(g0,g1)
  prefetch are emitted before group 6, so the i3 stream never drains;
  in-pass prefetch issues i3(g+2) at each group top.
- DMA issuance rings: SP (HWDGE) + Pool (SWDGE) alternating, with the
  Activation engine carrying ZERO stream DMA -- interleaved dma_starts
  on Act throttle its relus and cascade into the PE via the psum WAR
  chain (this alone was worth ~25us/iter). Relus alternate Act/DVE 1:1
  (4:3 Act-heavy measured as a wash once slot-position bias in the
  benchmarking rounds was accounted for).
"""

import ml_dtypes
import numpy as np

import concourse.bass as bass
import concourse.tile as tile
from concourse import bacc
from concourse import mybir
from concourse.bass_utils import run_bass_kernel_spmd

B = 64
L = 128
C = 16
FILTERS = 8
K = 5
N_CORES = 8

L1 = L - K + 1      # 124
L2 = L1 - K + 1     # 120
L3 = L2 - K + 1     # 116

C0 = C // N_CORES           # 2    input cols per core
C1 = C0 * FILTERS           # 16   conv1 out-ch per core
C2 = C1 * FILTERS           # 128  conv2 out-ch per core
C3 = C2 * FILTERS           # 1024 conv3 out-ch per core
G3 = C2 // C1               # 8    conv3 groups of 16 in-ch
BG = 8                      # batch groups
B8 = B // BG                # 8

NB1 = L1 * B8               # 992   conv1 free (l1, b8)
NB2 = L2 * B                # 7680  conv2 cols (bg, l2, b8)
SB3 = L2 * B8               # 960   per-bg col stride
VB3 = L3 * B8               # 928   valid cols per bg block

F32 = mybir.dt.float32
BF16 = mybir.dt.bfloat16

_CACHE = {}


def _build_nc(reps=1, mode='full'):
    """Build the SPMD Bass program (same on all 8 cores).

    reps>1 wraps the pipeline in a device-side loop (used only for
    timing by wall-clock differencing).
    """
    nc = bacc.Bacc("TRN2", target_bir_lowering=False, debug=False)

    # cs (f32): col0 b1p [(c1*8+bg)], col1 b2p [c2], cols 2..9 b3p per group
    # sb (bf16): s1 [80,128] @0, s2 [80,128] @128, s3 8x[80,128] @256
    a1 = nc.declare_dram_parameter("a1", [80, NB1], BF16, isOutput=False)
    cs = nc.declare_dram_parameter("cs", [128, 10], F32, isOutput=False)
    sb = nc.declare_dram_parameter("sb", [80, 256 + G3 * 128], BF16, isOutput=False)
    wt = nc.declare_dram_parameter("wt", [128, G3 * L3 * 2], BF16, isOutput=False)
    out = nc.declare_dram_parameter("out", [128, 4 * B], F32, isOutput=True)

    do_mm = mode not in ('empty', 'conv2stop', 'i3only', 'i3h', 'i3sp', 'i3sync', 'i3dram', 'i3tri', 'i3gp')
    do_relu3 = mode not in ('norelu',)
    do_dense = mode not in ('nodense',)

    with tile.TileContext(nc) as tc:
        with (
            tc.tile_pool(name="consts", bufs=1) as consts,
            tc.tile_pool(name="work", bufs=1) as work,
            tc.tile_pool(name="i3pool", bufs=16) as i3pool,
            tc.tile_pool(name="zpool", bufs=3) as zpool,
            tc.tile_pool(name="psum", bufs=3, space=bass.MemorySpace.PSUM) as psum,
            tc.tile_pool(name="psumd", bufs=1, space=bass.MemorySpace.PSUM) as psumd,
            tc.tile_pool(name="dspill", bufs=1, space="DRAM") as dspill,
        ):
            # ---- constants ----
            a1_t = consts.tile([80, NB1], BF16)
            cs_t = consts.tile([128, 10], F32)
            sb_t = consts.tile([80, 256 + G3 * 128], BF16)
            nc.sync.dma_start(a1_t[:], a1[:])
            nc.scalar.dma_start(cs_t[:], cs[:])
            nc.sync.dma_start(sb_t[:], sb[:])
            WQ = G3 * L3 * 2 // 4  # 464
            wt_ts = []
            for q in range(4):
                wq = consts.tile([128, WQ], BF16, tag=f"wt{q}")
                (nc.scalar if q % 2 else nc.sync).dma_start(
                    wq[:], wt[:, q * WQ:(q + 1) * WQ])
                wt_ts.append(wq)

            # persistent work tiles (single-buffered; WAR handled by sems)
            tmp1 = work.tile([128, NB1], BF16)
            i2 = work.tile([80, NB2], BF16)
            x2r = work.tile([C2, NB2], BF16, name="x2r")
            dram_i3 = mode in ('fulldram', 'i3dram')
            x2d = (dspill.tile([C2, NB2], BF16, name="x2d")
                   if dram_i3 else None)

            # DMA ring rotation: keep the Activation engine OUT of DMA
            # issuance (it runs half the relus; interleaved dma_starts
            # throttle it and cascade into the PE via the psum WAR chain).
            # SP (HWDGE) + Pool (SWDGE) carry the stream instead.
            rings = [nc.sync, nc.gpsimd]
            if mode in ('i3tri', 'fulltri'):
                rings = [nc.sync, nc.scalar, nc.gpsimd]
            elif mode == 'fullq':
                rings = [nc.sync, nc.scalar, nc.sync, nc.gpsimd]
            elif mode == 'oldrings':
                rings = [nc.sync, nc.scalar]
            elif mode == 'fullng3':
                rings = [nc.sync, nc.gpsimd, nc.sync]
            elif mode == 'fullq5':
                rings = [nc.sync, nc.gpsimd, nc.sync, nc.gpsimd, nc.scalar]
            elif mode in ('fullgp2',):
                rings = [nc.sync, nc.gpsimd, nc.gpsimd]
            elif mode in ('i3gp',):
                rings = [nc.gpsimd]
            elif mode == 'full65':
                rings = [nc.sync, nc.gpsimd] * 5 + [nc.sync]
            ring_i = [0]
            sync_only = mode in ('i3sync', 'fullsync')

            forced_ring = [None]

            def dma(dst, src):
                if forced_ring[0] is not None:
                    forced_ring[0].dma_start(dst, src)
                    return
                if sync_only:
                    nc.sync.dma_start(dst, src)
                    return
                rings[ring_i[0]].dma_start(dst, src)
                ring_i[0] = (ring_i[0] + 1) % len(rings)

            # relu engine alternation: Act / DVE (optionally Pool)
            relu_i = [0]
            n_relu_eng = 3 if mode == 'fullpool3' else 2
            pat = (0, 1, 0, 1, 0, 1, 0) if mode == 'relu43' else None

            def relu(dst, src, bias_ap):
                e = pat[relu_i[0] % 7] if pat else relu_i[0]
                if e == 0:
                    nc.scalar.activation(dst, src,
                                         mybir.ActivationFunctionType.Relu,
                                         bias=bias_ap)
                elif e == 1:
                    nc.vector.tensor_scalar(dst, src, bias_ap, 0.0,
                                            mybir.AluOpType.add,
                                            mybir.AluOpType.max)
                else:
                    nc.gpsimd.tensor_scalar(dst, src, bias_ap, 0.0,
                                            mybir.AluOpType.add,
                                            mybir.AluOpType.max)
                relu_i[0] = (relu_i[0] + 1) % (7 if pat else n_relu_eng)

            def prologue():
                """conv1 + i2 im2col + conv2 + issue i3(g0, g1)."""
                # conv1: 2 MMs into one 2-bank psum tile, one relu
                if mode != 'empty':
                    p1 = psum.tile([128, 1024], F32, tag="pchunk", name="p1")
                    nc.tensor.matmul(p1[:, 0:512], sb_t[0:80, 0:128],
                                     a1_t[:, 0:512], start=True, stop=True)
                    nc.tensor.matmul(p1[:, 512:NB1], sb_t[0:80, 0:128],
                                     a1_t[:, 512:NB1], start=True, stop=True)
                    relu(tmp1[:], p1[:, 0:NB1], cs_t[:, 0:1])

                    # i2 im2col: ONE DMA per bg. In fi/fb these ride
                    # the otherwise-idle Act ring: their only wait is on
                    # conv1-relu (often Act's own previous instruction),
                    # so they cannot wedge the relu stream.
                    for bg in range(BG):
                        t1b = tmp1[bg:128, 0:NB1]
                        src = bass.AP(t1b.tensor, t1b.offset,
                                      [[t1b.ap[0][0] * 8, C1], [B8, K],
                                       [1, SB3]])
                        d = i2[0:80, bg * SB3:(bg + 1) * SB3]
                        if mode in ('fi', 'fb'):
                            nc.scalar.dma_start(d, src)
                        else:
                            dma(d, src)

                    # conv2: 15 MMs, relu per 1024 (512 for the tail)
                    for t in range(8):
                        n = 1024 if t < 7 else 512
                        p2 = psum.tile([128, 1024], F32, tag="pchunk",
                                       name="p2")
                        nc.tensor.matmul(p2[:, 0:512], sb_t[0:80, 128:256],
                                         i2[:, t * 1024:t * 1024 + 512],
                                         start=True, stop=True)
                        if n == 1024:
                            nc.tensor.matmul(
                                p2[:, 512:1024], sb_t[0:80, 128:256],
                                i2[:, t * 1024 + 512:(t + 1) * 1024],
                                start=True, stop=True)
                        relu(x2r[:, t * 1024:t * 1024 + n], p2[:, 0:n],
                             cs_t[:, 1:2])
                    if dram_i3:
                        nc.sync.dma_start(x2d[:, 0:NB2 // 2],
                                          x2r[:, 0:NB2 // 2])
                        nc.scalar.dma_start(x2d[:, NB2 // 2:NB2],
                                            x2r[:, NB2 // 2:NB2])

            i3s = {}
            HB3 = 4 * VB3   # 3712: packed half-group block (4 bg x 928)

            def issue_i3(g):
                for h in range(2):
                    issue_i3_half(g, h)

            def issue_i3_half(g, h):
                # 10 DMAs per half (tap k x e); dst i3 rows (c*5+k), c=8e+d;
                # i3h[(c,k), bg*928 + col] = x2r[P(c), (4h+bg)*960 + col + 8k]
                i3 = i3pool.tile([128, HB3], BF16, tag="i3", name="i3")
                xb = (x2d if dram_i3 else x2r)[0:128, 0:NB2]
                xp = xb.ap[0][0]
                ib = i3[0:128, 0:HB3]
                pp = ib.ap[0][0]
                pbase = (g % 4) + 32 * (g // 4)
                for k in range(K):
                    for e in range(1 if mode == 'i3h' else 2):
                        src = bass.AP(xb.tensor,
                                      xb.offset + (pbase + 64 * e) * xp
                                      + 8 * k + h * 4 * SB3,
                                      [[4 * xp, 8], [SB3, 4], [1, VB3]])
                        dst = bass.AP(ib.tensor,
                                      ib.offset + (40 * e + k) * pp,
                                      [[5 * pp, 8], [VB3, 4], [1, VB3]])
                        if mode == 'ringrot':
                            dma(dst, src)
                        else:
                            # ring-by-e: each ring's transfers touch a
                            # disjoint src-partition half (e-halves are 64
                            # partitions apart), so the two rings never
                            # contend on the same SBUF source ports
                            (nc.sync if e == 0 else nc.gpsimd).dma_start(
                                dst, src)
                i3s[(g, h)] = i3

            # dense: quad-MMs (4 l x 2 actions = 8 psum rows, moving
            # cols (bg, 4, b8) = 256); 4 col-tile accumulators in ONE
            # psum bank (rows 32*tj..32*tj+8, cols 0..256).
            NQ = L3 // 4                 # 29 quads per group
            per_tile = G3 * NQ // 4      # 58 MMs per col-tile
            pd = psumd.tile([128, 512], F32, tag="pd", name="pd")
            tile_seen = [0, 0, 0, 0]
            qcount = [0]

            def emit_dense_quad(zf, g):
                lq = qcount[0] % NQ
                qcount[0] += 1
                zb = zf[0:128, 0:1]
                zp, zf0 = zb.ap[0], zb.offset
                tj = (g * NQ + lq) % 4
                wcol = 232 * (g % 2) + 8 * lq
                mv = bass.AP(zb.tensor, zf0 + 4 * lq * B8,
                             [zp, [VB3, BG], [B8, 4], [1, B8]])
                nc.tensor.matmul(pd[32 * tj:32 * tj + 8, 0:4 * B],
                                 wt_ts[g // 2][:, wcol:wcol + 8],
                                 mv,
                                 start=(tile_seen[tj] == 0),
                                 stop=(tile_seen[tj] == per_tile - 1),
                                 tile_position=(0, 32 * tj))
                tile_seen[tj] += 1

            def conv3_group(g, quads):
                """conv3 MMs + relu for group g; sprinkle the dense quads
                of `quads` (list of (zf, group)) across the bg slots,
                BEFORE each slot's chunk MMs so they fill PE bubbles."""
                if do_relu3:
                    zf = zpool.tile([C2, BG * VB3], BF16, tag="zf", name="zf")
                else:
                    zf = x2r  # junk moving data for the norelu timing probe
                qi = 0
                for bg in range(BG):
                    if do_dense:
                        # front-loaded: fills the i3-gate bubble early and
                        # finishes the pend group's zf reads sooner (frees
                        # its buffer for the zf WAR 3-buf rotation)
                        frac = (0, 22, 38, 52, 64, 76, 86, 94, 100)
                        nq = len(quads) * frac[bg + 1] // 100
                        while qi < nq:
                            emit_dense_quad(*quads[qi])
                            qi += 1
                    i3 = i3s[(g, bg // 4)]
                    p3 = psum.tile([128, 1024], F32, tag="pchunk", name="p3")
                    c0 = (bg % 4) * VB3
                    nc.tensor.matmul(p3[:, 0:512],
                                     sb_t[:, 256 + g * 128:256 + (g + 1) * 128],
                                     i3[0:80, c0:c0 + 512], start=True, stop=True)
                    nc.tensor.matmul(p3[:, 512:VB3],
                                     sb_t[:, 256 + g * 128:256 + (g + 1) * 128],
                                     i3[0:80, c0 + 512:c0 + VB3],
                                     start=True, stop=True)
                    if do_relu3:
                        relu(zf[:, bg * VB3:(bg + 1) * VB3], p3[:, 0:VB3],
                             cs_t[:, 2 + g:3 + g])
                i3s.pop((g, 0))
                i3s.pop((g, 1))
                return zf

            rep_cm = (tc.For_i(0, reps, 1,
                               hint_engines=(mybir.EngineType.PE,
                                             mybir.EngineType.DVE,
                                             mybir.EngineType.Activation,
                                             mybir.EngineType.SP,
                                             mybir.EngineType.Pool))
                      if reps > 1 else None)

            def dense_tail(zf, g):
                for _ in range(NQ):
                    emit_dense_quad(zf, g)

            def prologue_full(preloop=False):
                # next pass's x2r + 2-group i3 prefetch: the transfers keep
                # the DMA queues busy straight through the For_i barrier
                # (only sync is barriered, data flows via persistent tiles)
                prologue()
                if mode not in ('empty', 'conv2stop'):
                    ng = 2
                    if not preloop:
                        ng = {'fat': 1, 'fat2': 0}.get(mode, 2)
                    for g in range(ng):
                        issue_i3(g)

            prologue_full(preloop=True)
            if rep_cm is not None:
                rep_cm.__enter__()

            if do_mm:
                zfs = {}
                lead = 3 if mode == 'fog3' else 2
                if mode == 'withgate':
                    # one-time PE gate on a 2-group i3 head start
                    pscr = psum.tile([128, 1024], F32, tag="pchunk",
                                     name="pscr")
                    nc.tensor.matmul(pscr[0:1, 0:1], sb_t[0:80, 0:1],
                                     i3s[(1, 1)][0:80, 0:1],
                                     start=True, stop=True)
                for g in range(G3):
                    if g + lead < G3:
                        issue_i3(g + lead)
                    if g == 6 and rep_cm is not None:
                        prologue_full()
                    quads = []
                    if g >= 2:
                        quads += [(zfs.pop(g - 2), g - 2)] * NQ
                    if g == 7:
                        quads += [(zfs[6], 6)] * NQ
                    zfs[g] = conv3_group(g, quads)
                if rep_cm is not None and mode in ('fat', 'fat2'):
                    # deferred cross-barrier prefetch on the Act ring: Act
                    # has no relus left after group 7, the waits (x2r RAW,
                    # ancient buf WAR) are satisfied, and it adds ring
                    # bandwidth exactly in the tail/barrier/head window
                    forced_ring[0] = nc.scalar
                    for g in range({'fat': 1, 'fat2': 0}[mode], 2):
                        issue_i3(g)
                    forced_ring[0] = None
                if do_dense:
                    dense_tail(zfs[7], 7)
                zfs.clear()
            else:
                if mode in ('i3only', 'i3h', 'i3sp', 'i3sync', 'i3dram', 'i3tri', 'i3gp'):
                    for g in range(6):
                        issue_i3(g + 2)
                    i3s.clear()
                if rep_cm is not None:
                    prologue_full()

            # ---- write partials ----
            out_t = work.tile([128, 4 * B], F32)
            if mode in ('full', 'outact'):
                for tj in range(4):
                    nc.vector.tensor_copy(out_t[32 * tj:32 * tj + 8, :],
                                          pd[32 * tj:32 * tj + 8, 0:4 * B])
            else:
                nc.gpsimd.memset(out_t[:], 0.0)
            out_ring = nc.scalar if mode == 'outact' else nc.sync
            out_ring.dma_start(out[:], out_t[:])

            if rep_cm is not None:
                rep_cm.__exit__(None, None, None)

    nc.compile()
    return nc


def _shard_inputs(state, k1, b1, k2, b2, k3, b3, W, bd):
    """Host-side: build per-core input maps (layout only, no math)."""
    state = np.asarray(state, dtype=np.float32)
    k1 = np.asarray(k1, np.float32); b1 = np.asarray(b1, np.float32)
    k2 = np.asarray(k2, np.float32); b2 = np.asarray(b2, np.float32)
    k3 = np.asarray(k3, np.float32); b3 = np.asarray(b3, np.float32)
    W = np.asarray(W, np.float32)
    W3 = W.reshape(L3, C3 * N_CORES, 2)

    in_maps = []
    for j in range(N_CORES):
        x0 = state[:, :, C0 * j:C0 * (j + 1)]  # [B, L, 2]

        # conv1 im2col a1 [80=(bg,k,c0), (l1, b8)] bf16
        a1 = np.zeros((80, NB1), np.float32)
        for bg in range(BG):
            for k in range(K):
                for c in range(C0):
                    a1[bg * 10 + k * C0 + c] = (
                        x0[bg * B8:(bg + 1) * B8, k:k + L1, c].T.reshape(-1))

        # conv1 stationary blockdiag [80, 128=(c1*8+bg)]
        s1 = np.zeros((80, 128), np.float32)
        for bg in range(BG):
            for c in range(C0):
                for k in range(K):
                    for f in range(FILTERS):
                        c1i = c * FILTERS + f
                        s1[bg * 10 + k * C0 + c,
                           c1i * 8 + bg] = k1[k, 0, (C0 * j + c) * FILTERS + f]
        b1p = np.zeros(128, np.float32)
        for c1i in range(C1):
            b1p[c1i * 8:(c1i + 1) * 8] = b1[C1 * j + c1i]

        # conv2 stationary [80=(c1*5+k), 128 = P(c2)]: out-channel
        # c2 = 8*cc+gg sits at partition P = 4*(cc%8)+(gg%4)+32*(gg//4)
        # +64*(cc//8) so each conv3 group's reads span all 16 ports.
        P2 = np.zeros(128, np.int64)
        for c2i in range(128):
            gg, cc = c2i % 8, c2i // 8
            P2[c2i] = 4 * (cc % 8) + (gg % 4) + 32 * (gg // 4) + 64 * (cc // 8)
        s2 = np.zeros((80, 128), np.float32)
        for k in range(K):
            for c in range(C1):
                for f in range(FILTERS):
                    s2[c * K + k, P2[c * FILTERS + f]] = (
                        k2[k, 0, (C1 * j + c) * FILTERS + f])
        b2p = np.zeros(128, np.float32)
        b2p[P2] = b2[C2 * j:C2 * (j + 1)]

        # conv3 stationaries [80=(c*5+k), 8x128]; group g = {c2: c2%8==g},
        # within-group out col m = c*8+f for c2 = 8c+g
        s3 = np.zeros((80, G3 * 128), np.float32)
        b3p = np.zeros((128, G3), np.float32)
        for g in range(G3):
            for c in range(C1):
                c2l = 8 * c + g
                for k in range(K):
                    for f in range(FILTERS):
                        s3[c * K + k, g * 128 + c * FILTERS + f] = (
                            k3[k, 0, (C2 * j + c2l) * FILTERS + f])
                b3p[c * FILTERS:(c + 1) * FILTERS, g] = b3[
                    (C2 * j + c2l) * FILTERS:(C2 * j + c2l) * FILTERS + FILTERS]

        cs = np.zeros((128, 10), np.float32)
        cs[:, 0] = b1p
        cs[:, 1] = b2p
        cs[:, 2:10] = b3p

        sbm = np.zeros((80, 256 + G3 * 128), np.float32)
        sbm[:, 0:128] = s1
        sbm[:, 128:256] = s2
        sbm[:, 256:] = s3

        # dense weights [128p = m(c,f), (g, lq, q, a)] bf16
        Wj = W3[:, C3 * j:C3 * (j + 1), :]          # [L3, 1024, 2]
        wtm = np.zeros((128, G3, L3 // 4, 4, 2), np.float32)
        for g in range(G3):
            for c in range(C1):
                c2l = 8 * c + g
                for f in range(FILTERS):
                    m = c * FILTERS + f
                    wtm[m, g] = Wj[:, c2l * FILTERS + f, :].reshape(L3 // 4, 4, 2)
        wt = wtm.reshape(128, G3 * L3 * 2).astype(ml_dtypes.bfloat16)

        in_maps.append({"a1": a1.astype(ml_dtypes.bfloat16),
                        "cs": cs, "wt": wt,
                        "sb": sbm.astype(ml_dtypes.bfloat16)})
    return in_maps


def kernel(state, k1, b1, k2, b2, k3, b3, W, bd, **run_kwargs):
    if "nc" not in _CACHE:
        _CACHE["nc"] = _build_nc()
    nc = _CACHE["nc"]
    in_maps = _shard_inputs(state, k1, b1, k2, b2, k3, b3, W, bd)
    res = run_bass_kernel_spmd(nc, in_maps, list(range(N_CORES)), **run_kwargs)
    # device out [128, 256]: cols are (bg 8, q 4, b8 8);
    # partial[a, bg*8+b8] = sum_{tj,q} out[32*tj + 2*q + a, bg*32 + q*8 + b8]
    total = np.zeros((2, B), np.float32)
    for c in range(N_CORES):
        o = np.asarray(res.results[c]["out"]).reshape(128, BG, 4, B8)
        for tj in range(4):
            for q in range(4):
                total += o[32 * tj + 2 * q:32 * tj + 2 * q + 2, :, q, :].reshape(2, B)
    out = np.tanh(total.T + np.asarray(bd, np.float32)).astype(np.float32)
    if run_kwargs.get("trace"):
        _CACHE["last_result"] = res
    return out


# revision 57
# speedup vs baseline: 1.0012x; 1.0012x over previous
"""Trainium2 Bass kernel for nn_Actor (3 grouped conv1d blocks + dense + tanh).

Sharding: column-parallel across 8 cores. Core j owns input channels
{2j, 2j+1}; every conv is grouped (depthwise x8 filters), so that
slice owns contiguous channel blocks through the whole net:
  conv1 out-ch [16j,16j+16), conv2 out-ch [128j,128j+128),
  conv3 out-ch [1024j, 1024j+1024), and rows {l*8192 + ch} of W.
Each core computes partial dense outputs; the host sums them, adds bd
and applies tanh.

Pipeline design (all bf16 compute, f32 psum):
- conv1 out tmp1 [128p = (c1*8 + bg), (l1, b8)].
- i2 [80p = (c1*5 + k), (bg, l2, b8)] via 8 DMAs (one per bg).
- x2r [128p = P2(c2), (bg, l2, b8)] 960-stride cols.
- conv3 im2col as half-group tiles i3 [80p = (c*5+k), 4x928]:
  i3[(c,k), bg*928+col] = x2r[P(c), bg*960 + col + 8k] (tap shift is a
  column offset, so each (partition, tap, bg) is one 1856B contiguous
  segment; 10 DMA instructions per half, 16 tile bufs so every issue's
  buffer WAR is ancient and the issuing rings never block; the e=0 half
  rides the SP ring and e=1 the Pool ring, so concurrent transfers never
  contend on the same SBUF source ports).
- conv3: per (group, bg): 2 MMs (512+416) into a [128, 1024] 2-bank psum
  tile (3-buf rotation), ONE relu [128, 928] per tile (alternating
  Act/DVE) -> packed zf [128, 8*928] (bufs=3).
- dense: quad-MMs reading zf(g-2) (2-group-old => RAW waits ancient),
  interleaved ~4 per bg slot BEFORE the chunk MMs so PE gate-waits are
  covered by wait-free MMs; 4 col-tile accumulators in ONE psum bank;
  group 6's quads fold into group 7's slots, only group 7 tails.
- per-iteration software pipelining (For_i has an all-engine barrier, but
  DMA queues run through it): the NEXT pass's conv1/i2/conv2 + i3(g0,g1)
  prefetch are emitted before group 6, so the i3 stream never drains;
  in-pass prefetch issues i3(g+2) at each group top.
- DMA issuance rings: SP (HWDGE) + Pool (SWDGE) alternating, with the
  Activation engine carrying ZERO stream DMA -- interleaved dma_starts
  on Act throttle its relus and cascade into the PE via the psum WAR
  chain (this alone was worth ~25us/iter). Relus alternate Act/DVE 1:1
  (4:3 Act-heavy measured as a wash once slot-position bias in the
  benchmarking rounds was accounted for).
"""

import ml_dtypes
import numpy as np

import concourse.bass as bass
import concourse.tile as tile
from concourse import bacc
from concourse import mybir
from concourse.bass_utils import run_bass_kernel_spmd

B = 64
L = 128
C = 16
FILTERS = 8
K = 5
N_CORES = 8

L1 = L - K + 1      # 124
L2 = L1 - K + 1     # 120
L3 = L2 - K + 1     # 116

C0 = C // N_CORES           # 2    input cols per core
C1 = C0 * FILTERS           # 16   conv1 out-ch per core
C2 = C1 * FILTERS           # 128  conv2 out-ch per core
C3 = C2 * FILTERS           # 1024 conv3 out-ch per core
G3 = C2 // C1               # 8    conv3 groups of 16 in-ch
BG = 8                      # batch groups
B8 = B // BG                # 8

NB1 = L1 * B8               # 992   conv1 free (l1, b8)
NB2 = L2 * B                # 7680  conv2 cols (bg, l2, b8)
SB3 = L2 * B8               # 960   per-bg col stride
VB3 = L3 * B8               # 928   valid cols per bg block

F32 = mybir.dt.float32
BF16 = mybir.dt.bfloat16

_CACHE = {}


def _build_nc(reps=1, mode='full'):
    """Build the SPMD Bass program (same on all 8 cores).

    reps>1 wraps the pipeline in a device-side loop (used only for
    timing by wall-clock differencing).
    """
    nc = bacc.Bacc("TRN2", target_bir_lowering=False, debug=False)

    # cs (f32): col0 b1p [(c1*8+bg)], col1 b2p [c2], cols 2..9 b3p per group
    # sb (bf16): s1 [80,128] @0, s2 [80,128] @128, s3 8x[80,128] @256
    a1 = nc.declare_dram_parameter("a1", [80, NB1], BF16, isOutput=False)
    cs = nc.declare_dram_parameter("cs", [128, 10], F32, isOutput=False)
    sb = nc.declare_dram_parameter("sb", [80, 256 + G3 * 128], BF16, isOutput=False)
    wt = nc.declare_dram_parameter("wt", [128, G3 * L3 * 2], BF16, isOutput=False)
    out = nc.declare_dram_parameter("out", [128, 4 * B], F32, isOutput=True)

    do_mm = mode not in ('empty', 'conv2stop', 'i3only', 'i3h', 'i3sp', 'i3sync', 'i3dram', 'i3tri', 'i3gp')
    do_relu3 = mode not in ('norelu',)
    do_dense = mode not in ('nodense',)

    with tile.TileContext(nc) as tc:
        with (
            tc.tile_pool(name="consts", bufs=1) as consts,
            tc.tile_pool(name="work", bufs=1) as work,
            tc.tile_pool(name="i3pool", bufs=16) as i3pool,
            tc.tile_pool(name="zpool", bufs=3) as zpool,
            tc.tile_pool(name="psum", bufs=3, space=bass.MemorySpace.PSUM) as psum,
            tc.tile_pool(name="psumd", bufs=1, space=bass.MemorySpace.PSUM) as psumd,
            tc.tile_pool(name="dspill", bufs=1, space="DRAM") as dspill,
        ):
            # ---- constants ----
            a1_t = consts.tile([80, NB1], BF16)
            cs_t = consts.tile([128, 10], F32)
            sb_t = consts.tile([80, 256 + G3 * 128], BF16)
            nc.sync.dma_start(a1_t[:], a1[:])
            nc.scalar.dma_start(cs_t[:], cs[:])
            nc.sync.dma_start(sb_t[:], sb[:])
            WQ = G3 * L3 * 2 // 4  # 464
            wt_ts = []
            for q in range(4):
                wq = consts.tile([128, WQ], BF16, tag=f"wt{q}")
                (nc.scalar if q % 2 else nc.sync).dma_start(
                    wq[:], wt[:, q * WQ:(q + 1) * WQ])
                wt_ts.append(wq)

            # persistent work tiles (single-buffered; WAR handled by sems)
            tmp1 = work.tile([128, NB1], BF16)
            i2 = work.tile([80, NB2], BF16)
            x2r = work.tile([C2, NB2], BF16, name="x2r")
            dram_i3 = mode in ('fulldram', 'i3dram')
            x2d = (dspill.tile([C2, NB2], BF16, name="x2d")
                   if dram_i3 else None)

            # DMA ring rotation: keep the Activation engine OUT of DMA
            # issuance (it runs half the relus; interleaved dma_starts
            # throttle it and cascade into the PE via the psum WAR chain).
            # SP (HWDGE) + Pool (SWDGE) carry the stream instead.
            rings = [nc.sync, nc.gpsimd]
            if mode in ('i3tri', 'fulltri'):
                rings = [nc.sync, nc.scalar, nc.gpsimd]
            elif mode == 'fullq':
                rings = [nc.sync, nc.scalar, nc.sync, nc.gpsimd]
            elif mode == 'oldrings':
                rings = [nc.sync, nc.scalar]
            elif mode == 'fullng3':
                rings = [nc.sync, nc.gpsimd, nc.sync]
            elif mode == 'fullq5':
                rings = [nc.sync, nc.gpsimd, nc.sync, nc.gpsimd, nc.scalar]
            elif mode in ('fullgp2',):
                rings = [nc.sync, nc.gpsimd, nc.gpsimd]
            elif mode in ('i3gp',):
                rings = [nc.gpsimd]
            elif mode == 'full65':
                rings = [nc.sync, nc.gpsimd] * 5 + [nc.sync]
            ring_i = [0]
            sync_only = mode in ('i3sync', 'fullsync')

            forced_ring = [None]

            def dma(dst, src):
                if forced_ring[0] is not None:
                    forced_ring[0].dma_start(dst, src)
                    return
                if sync_only:
                    nc.sync.dma_start(dst, src)
                    return
                rings[ring_i[0]].dma_start(dst, src)
                ring_i[0] = (ring_i[0] + 1) % len(rings)

            # relu engine alternation: Act / DVE (optionally Pool)
            relu_i = [0]
            n_relu_eng = 3 if mode == 'fullpool3' else 2
            pat = (0, 1, 0, 1, 0, 1, 0) if mode == 'relu43' else None

            def relu(dst, src, bias_ap):
                e = pat[relu_i[0] % 7] if pat else relu_i[0]
                if e == 0:
                    nc.scalar.activation(dst, src,
                                         mybir.ActivationFunctionType.Relu,
                                         bias=bias_ap)
                elif e == 1:
                    nc.vector.tensor_scalar(dst, src, bias_ap, 0.0,
                                            mybir.AluOpType.add,
                                            mybir.AluOpType.max)
                else:
                    nc.gpsimd.tensor_scalar(dst, src, bias_ap, 0.0,
                                            mybir.AluOpType.add,
                                            mybir.AluOpType.max)
                relu_i[0] = (relu_i[0] + 1) % (7 if pat else n_relu_eng)

            def prologue():
                """conv1 + i2 im2col + conv2 + issue i3(g0, g1)."""
                # conv1: 2 MMs into one 2-bank psum tile, one relu
                if mode != 'empty':
                    p1 = psum.tile([128, 1024], F32, tag="pchunk", name="p1")
                    nc.tensor.matmul(p1[:, 0:512], sb_t[0:80, 0:128],
                                     a1_t[:, 0:512], start=True, stop=True)
                    nc.tensor.matmul(p1[:, 512:NB1], sb_t[0:80, 0:128],
                                     a1_t[:, 512:NB1], start=True, stop=True)
                    relu(tmp1[:], p1[:, 0:NB1], cs_t[:, 0:1])

                    # i2 im2col: ONE DMA per bg. In fi/fb these ride
                    # the otherwise-idle Act ring: their only wait is on
                    # conv1-relu (often Act's own previous instruction),
                    # so they cannot wedge the relu stream.
                    for bg in range(BG):
                        t1b = tmp1[bg:128, 0:NB1]
                        src = bass.AP(t1b.tensor, t1b.offset,
                                      [[t1b.ap[0][0] * 8, C1], [B8, K],
                                       [1, SB3]])
                        d = i2[0:80, bg * SB3:(bg + 1) * SB3]
                        if mode in ('fi', 'fb'):
                            nc.scalar.dma_start(d, src)
                        else:
                            dma(d, src)

                    # conv2: 15 MMs, relu per 1024 (512 for the tail)
                    for t in range(8):
                        n = 1024 if t < 7 else 512
                        p2 = psum.tile([128, 1024], F32, tag="pchunk",
                                       name="p2")
                        nc.tensor.matmul(p2[:, 0:512], sb_t[0:80, 128:256],
                                         i2[:, t * 1024:t * 1024 + 512],
                                         start=True, stop=True)
                        if n == 1024:
                            nc.tensor.matmul(
                                p2[:, 512:1024], sb_t[0:80, 128:256],
                                i2[:, t * 1024 + 512:(t + 1) * 1024],
                                start=True, stop=True)
                        relu(x2r[:, t * 1024:t * 1024 + n], p2[:, 0:n],
                             cs_t[:, 1:2])
                    if dram_i3:
                        nc.sync.dma_start(x2d[:, 0:NB2 // 2],
                                          x2r[:, 0:NB2 // 2])
                        nc.scalar.dma_start(x2d[:, NB2 // 2:NB2],
                                            x2r[:, NB2 // 2:NB2])

            i3s = {}
            HB3 = 4 * VB3   # 3712: packed half-group block (4 bg x 928)

            def issue_i3(g):
                for h in range(2):
                    issue_i3_half(g, h)

            def issue_i3_half(g, h):
                # 10 DMAs per half (tap k x e); dst i3 rows (c*5+k), c=8e+d;
                # i3h[(c,k), bg*928 + col] = x2r[P(c), (4h+bg)*960 + col + 8k]
                i3 = i3pool.tile([128, HB3], BF16, tag="i3", name="i3")
                xb = (x2d if dram_i3 else x2r)[0:128, 0:NB2]
                xp = xb.ap[0][0]
                ib = i3[0:128, 0:HB3]
                pp = ib.ap[0][0]
                pbase = (g % 4) + 32 * (g // 4)
                for k in range(K):
                    for e in range(1 if mode == 'i3h' else 2):
                        src = bass.AP(xb.tensor,
                                      xb.offset + (pbase + 64 * e) * xp
                                      + 8 * k + h * 4 * SB3,
                                      [[4 * xp, 8], [SB3, 4], [1, VB3]])
                        dst = bass.AP(ib.tensor,
                                      ib.offset + (40 * e + k) * pp,
                                      [[5 * pp, 8], [VB3, 4], [1, VB3]])
                        if mode == 'ringrot':
                            dma(dst, src)
                        else:
                            # ring-by-e: each ring's transfers touch a
                            # disjoint src-partition half (e-halves are 64
                            # partitions apart), so the two rings never
                            # contend on the same SBUF source ports
                            (nc.sync if e == 0 else nc.gpsimd).dma_start(
                                dst, src)
                i3s[(g, h)] = i3

            # dense: quad-MMs (4 l x 2 actions = 8 psum rows, moving
            # cols (bg, 4, b8) = 256); 4 col-tile accumulators in ONE
            # psum bank (rows 32*tj..32*tj+8, cols 0..256).
            NQ = L3 // 4                 # 29 quads per group
            per_tile = G3 * NQ // 4      # 58 MMs per col-tile
            pd = psumd.tile([128, 512], F32, tag="pd", name="pd")
            tile_seen = [0, 0, 0, 0]
            qcount = [0]

            def emit_dense_quad(zf, g):
                lq = qcount[0] % NQ
                qcount[0] += 1
                zb = zf[0:128, 0:1]
                zp, zf0 = zb.ap[0], zb.offset
                tj = (g * NQ + lq) % 4
                wcol = 232 * (g % 2) + 8 * lq
                mv = bass.AP(zb.tensor, zf0 + 4 * lq * B8,
                             [zp, [VB3, BG], [B8, 4], [1, B8]])
                nc.tensor.matmul(pd[32 * tj:32 * tj + 8, 0:4 * B],
                                 wt_ts[g // 2][:, wcol:wcol + 8],
                                 mv,
                                 start=(tile_seen[tj] == 0),
                                 stop=(tile_seen[tj] == per_tile - 1),
                                 tile_position=(0, 32 * tj))
                tile_seen[tj] += 1

            def conv3_group(g, quads):
                """conv3 MMs + relu for group g; sprinkle the dense quads
                of `quads` (list of (zf, group)) across the bg slots,
                BEFORE each slot's chunk MMs so they fill PE bubbles."""
                if do_relu3:
                    zf = zpool.tile([C2, BG * VB3], BF16, tag="zf", name="zf")
                else:
                    zf = x2r  # junk moving data for the norelu timing probe
                qi = 0
                for bg in range(BG):
                    if do_dense:
                        # uniform spread of the pend group's quads across
                        # the bg slots (front-loading A/B'd worse)
                        if mode == 'quadf':
                            frac = (0, 22, 38, 52, 64, 76, 86, 94, 100)
                        else:
                            frac = (0, 13, 25, 38, 50, 63, 75, 88, 100)
                        nq = len(quads) * frac[bg + 1] // 100
                        while qi < nq:
                            emit_dense_quad(*quads[qi])
                            qi += 1
                    i3 = i3s[(g, bg // 4)]
                    p3 = psum.tile([128, 1024], F32, tag="pchunk", name="p3")
                    c0 = (bg % 4) * VB3
                    nc.tensor.matmul(p3[:, 0:512],
                                     sb_t[:, 256 + g * 128:256 + (g + 1) * 128],
                                     i3[0:80, c0:c0 + 512], start=True, stop=True)
                    nc.tensor.matmul(p3[:, 512:VB3],
                                     sb_t[:, 256 + g * 128:256 + (g + 1) * 128],
                                     i3[0:80, c0 + 512:c0 + VB3],
                                     start=True, stop=True)
                    if do_relu3:
                        relu(zf[:, bg * VB3:(bg + 1) * VB3], p3[:, 0:VB3],
                             cs_t[:, 2 + g:3 + g])
                i3s.pop((g, 0))
                i3s.pop((g, 1))
                return zf

            rep_cm = (tc.For_i(0, reps, 1,
                               hint_engines=(mybir.EngineType.PE,
                                             mybir.EngineType.DVE,
                                             mybir.EngineType.Activation,
                                             mybir.EngineType.SP,
                                             mybir.EngineType.Pool))
                      if reps > 1 else None)

            def dense_tail(zf, g):
                for _ in range(NQ):
                    emit_dense_quad(zf, g)

            def prologue_full(preloop=False):
                # next pass's x2r + 2-group i3 prefetch: the transfers keep
                # the DMA queues busy straight through the For_i barrier
                # (only sync is barriered, data flows via persistent tiles)
                prologue()
                if mode not in ('empty', 'conv2stop'):
                    ng = 2
                    if not preloop:
                        ng = {'fat': 1, 'fat2': 0}.get(mode, 2)
                    for g in range(ng):
                        issue_i3(g)

            prologue_full(preloop=True)
            if rep_cm is not None:
                rep_cm.__enter__()

            if do_mm:
                zfs = {}
                lead = 3 if mode == 'fog3' else 2
                if mode == 'withgate':
                    # one-time PE gate on a 2-group i3 head start
                    pscr = psum.tile([128, 1024], F32, tag="pchunk",
                                     name="pscr")
                    nc.tensor.matmul(pscr[0:1, 0:1], sb_t[0:80, 0:1],
                                     i3s[(1, 1)][0:80, 0:1],
                                     start=True, stop=True)
                for g in range(G3):
                    if g + lead < G3:
                        issue_i3(g + lead)
                    if g == 6 and rep_cm is not None:
                        prologue_full()
                    quads = []
                    if g >= 2:
                        quads += [(zfs.pop(g - 2), g - 2)] * NQ
                    if g == 7:
                        quads += [(zfs[6], 6)] * NQ
                    zfs[g] = conv3_group(g, quads)
                if rep_cm is not None and mode in ('fat', 'fat2'):
                    # deferred cross-barrier prefetch on the Act ring: Act
                    # has no relus left after group 7, the waits (x2r RAW,
                    # ancient buf WAR) are satisfied, and it adds ring
                    # bandwidth exactly in the tail/barrier/head window
                    forced_ring[0] = nc.scalar
                    for g in range({'fat': 1, 'fat2': 0}[mode], 2):
                        issue_i3(g)
                    forced_ring[0] = None
                if do_dense:
                    dense_tail(zfs[7], 7)
                zfs.clear()
            else:
                if mode in ('i3only', 'i3h', 'i3sp', 'i3sync', 'i3dram', 'i3tri', 'i3gp'):
                    for g in range(6):
                        issue_i3(g + 2)
                    i3s.clear()
                if rep_cm is not None:
                    prologue_full()

            # ---- write partials ----
            out_t = work.tile([128, 4 * B], F32)
            if mode in ('full', 'outact'):
                for tj in range(4):
                    nc.vector.tensor_copy(out_t[32 * tj:32 * tj + 8, :],
                                          pd[32 * tj:32 * tj + 8, 0:4 * B])
            else:
                nc.gpsimd.memset(out_t[:], 0.0)
            out_ring = nc.scalar if mode == 'outact' else nc.sync
            out_ring.dma_start(out[:], out_t[:])

            if rep_cm is not None:
                rep_cm.__exit__(None, None, None)

    nc.compile()
    return nc


def _shard_inputs(state, k1, b1, k2, b2, k3, b3, W, bd):
    """Host-side: build per-core input maps (layout only, no math)."""
    state = np.asarray(state, dtype=np.float32)
    k1 = np.asarray(k1, np.float32); b1 = np.asarray(b1, np.float32)
    k2 = np.asarray(k2, np.float32); b2 = np.asarray(b2, np.float32)
    k3 = np.asarray(k3, np.float32); b3 = np.asarray(b3, np.float32)
    W = np.asarray(W, np.float32)
    W3 = W.reshape(L3, C3 * N_CORES, 2)

    in_maps = []
    for j in range(N_CORES):
        x0 = state[:, :, C0 * j:C0 * (j + 1)]  # [B, L, 2]

        # conv1 im2col a1 [80=(bg,k,c0), (l1, b8)] bf16
        a1 = np.zeros((80, NB1), np.float32)
        for bg in range(BG):
            for k in range(K):
                for c in range(C0):
                    a1[bg * 10 + k * C0 + c] = (
                        x0[bg * B8:(bg + 1) * B8, k:k + L1, c].T.reshape(-1))

        # conv1 stationary blockdiag [80, 128=(c1*8+bg)]
        s1 = np.zeros((80, 128), np.float32)
        for bg in range(BG):
            for c in range(C0):
                for k in range(K):
                    for f in range(FILTERS):
                        c1i = c * FILTERS + f
                        s1[bg * 10 + k * C0 + c,
                           c1i * 8 + bg] = k1[k, 0, (C0 * j + c) * FILTERS + f]
        b1p = np.zeros(128, np.float32)
        for c1i in range(C1):
            b1p[c1i * 8:(c1i + 1) * 8] = b1[C1 * j + c1i]

        # conv2 stationary [80=(c1*5+k), 128 = P(c2)]: out-channel
        # c2 = 8*cc+gg sits at partition P = 4*(cc%8)+(gg%4)+32*(gg//4)
        # +64*(cc//8) so each conv3 group's reads span all 16 ports.
        P2 = np.zeros(128, np.int64)
        for c2i in range(128):
            gg, cc = c2i % 8, c2i // 8
            P2[c2i] = 4 * (cc % 8) + (gg % 4) + 32 * (gg // 4) + 64 * (cc // 8)
        s2 = np.zeros((80, 128), np.float32)
        for k in range(K):
            for c in range(C1):
                for f in range(FILTERS):
                    s2[c * K + k, P2[c * FILTERS + f]] = (
                        k2[k, 0, (C1 * j + c) * FILTERS + f])
        b2p = np.zeros(128, np.float32)
        b2p[P2] = b2[C2 * j:C2 * (j + 1)]

        # conv3 stationaries [80=(c*5+k), 8x128]; group g = {c2: c2%8==g},
        # within-group out col m = c*8+f for c2 = 8c+g
        s3 = np.zeros((80, G3 * 128), np.float32)
        b3p = np.zeros((128, G3), np.float32)
        for g in range(G3):
            for c in range(C1):
                c2l = 8 * c + g
                for k in range(K):
                    for f in range(FILTERS):
                        s3[c * K + k, g * 128 + c * FILTERS + f] = (
                            k3[k, 0, (C2 * j + c2l) * FILTERS + f])
                b3p[c * FILTERS:(c + 1) * FILTERS, g] = b3[
                    (C2 * j + c2l) * FILTERS:(C2 * j + c2l) * FILTERS + FILTERS]

        cs = np.zeros((128, 10), np.float32)
        cs[:, 0] = b1p
        cs[:, 1] = b2p
        cs[:, 2:10] = b3p

        sbm = np.zeros((80, 256 + G3 * 128), np.float32)
        sbm[:, 0:128] = s1
        sbm[:, 128:256] = s2
        sbm[:, 256:] = s3

        # dense weights [128p = m(c,f), (g, lq, q, a)] bf16
        Wj = W3[:, C3 * j:C3 * (j + 1), :]          # [L3, 1024, 2]
        wtm = np.zeros((128, G3, L3 // 4, 4, 2), np.float32)
        for g in range(G3):
            for c in range(C1):
                c2l = 8 * c + g
                for f in range(FILTERS):
                    m = c * FILTERS + f
                    wtm[m, g] = Wj[:, c2l * FILTERS + f, :].reshape(L3 // 4, 4, 2)
        wt = wtm.reshape(128, G3 * L3 * 2).astype(ml_dtypes.bfloat16)

        in_maps.append({"a1": a1.astype(ml_dtypes.bfloat16),
                        "cs": cs, "wt": wt,
                        "sb": sbm.astype(ml_dtypes.bfloat16)})
    return in_maps


def kernel(state, k1, b1, k2, b2, k3, b3, W, bd, **run_kwargs):
    if "nc" not in _CACHE:
        _CACHE["nc"] = _build_nc()
    nc = _CACHE["nc"]
    in_maps = _shard_inputs(state, k1, b1, k2, b2, k3, b3, W, bd)
    res = run_bass_kernel_spmd(nc, in_maps, list(range(N_CORES)), **run_kwargs)
    # device out [128, 256]: cols are (bg 8, q 4, b8 8);
    # partial[a, bg*8+b8] = sum_{tj,q} out[32*tj + 2*q + a, bg*32 + q*8 + b8]
    total = np.zeros((2, B), np.float32)
    for c in range(N_CORES):
        o = np.asarray(res.results[c]["out"]).reshape(128, BG, 4, B8)
        for tj in range(4):
            for q in range(4):
                total += o[32 * tj + 2 * q:32 * tj + 2 * q + 2, :, q, :].reshape(2, B)
    out = np.tanh(total.T + np.asarray(bd, np.float32)).astype(np.float32)
    if run_kwargs.get("trace"):
        _CACHE["last_result"] = res
    return out
